# revision 31
# baseline (speedup 1.0000x reference)
"""DGCNN transform-net forward on 8 TRN2 NeuronCores (Bass/Tile).

Sharding: cores 2b, 2b+1 handle cloud b; each computes knn/features for half
the cloud (local rows 0..2047 in a rotated frame). Pair AllGather per edge
block exchanges feature halves; 8-way AllReduce combines BN statistics.

Per edge block: h_ij = u_j + v_i with u = Wd x, v = (Wc-Wd) x + b.
Exact top-40 by distance via 32-segment top-8 + candidate rounds (DVE),
per-slot indirect-DMA gathers of u rows, DVE strided max-reduce, and
PE accumulation matmuls for the BN moments. max-over-k commutes with the
BN affine + leaky relu (positive scale), so only per-point maxima are kept.
"""
import sys
sys.path.insert(0, '/opt/trn_rl_repo')
import math
import numpy as np
import concourse.bass as bass
import concourse.mybir as mybir
from concourse import bacc
from concourse.tile import TileContext
from concourse.masks import make_identity

f32 = mybir.dt.float32
i16 = mybir.dt.int16
u16 = mybir.dt.uint16
i32 = mybir.dt.int32
AX = mybir.AxisListType
OP = mybir.AluOpType
ACTF = mybir.ActivationFunctionType

B, N, K = 4, 4096, 40
NH = N // 2
NT = NH // 128
SEG, SL = 32, 128
NCAND = SEG * 8
BLOCKS = [(3, 64), (64, 64), (64, 128), (128, 256)]
EPS = 1e-5
PAIRS = [[0, 1], [2, 3], [4, 5], [6, 7]]
ALL8 = [list(range(8))]
SQ2 = math.sqrt(2.0)


def _chan(nc, pool, ext, tiles, tag):
    t = pool.tile([128, tiles], f32, tag=tag)
    dim = ext.shape[0]
    if dim >= 128:
        nc.sync.dma_start(out=t[:, :], in_=ext.rearrange("(t p) -> p t", p=128))
    else:
        nc.sync.dma_start(out=t[:dim, 0:1], in_=ext[:, None])
    return t


def build():
    nc = bacc.Bacc("TRN2", debug=False)

    x_ext = nc.dram_tensor("x0", [3, N], f32, kind="ExternalInput")
    wdx, wvx, bbx, ggx, eex = [], [], [], [], []
    for li, (ci, co) in enumerate(BLOCKS):
        wdx.append(nc.dram_tensor(f"wd{li}", [ci, co], f32, kind="ExternalInput"))
        wvx.append(nc.dram_tensor(f"wv{li}", [ci, co], f32, kind="ExternalInput"))
        bbx.append(nc.dram_tensor(f"bb{li}", [co], f32, kind="ExternalInput"))
        ggx.append(nc.dram_tensor(f"gg{li}", [co], f32, kind="ExternalInput"))
        eex.append(nc.dram_tensor(f"ee{li}", [co], f32, kind="ExternalInput"))
    w5_ext = nc.dram_tensor("w5T", [512, 1024], f32, kind="ExternalInput")
    tw_ext = nc.dram_tensor("twT", [1024, 1024], f32, kind="ExternalInput")
    vecs = {}
    for nm, dim in (("b5", 1024), ("g5", 1024), ("e5", 1024), ("tb", 1024),
                    ("tg", 1024), ("te", 1024), ("f1b", 512), ("f1g", 512),
                    ("f1e", 512), ("f2b", 256), ("f2g", 256), ("f2e", 256),
                    ("f3b", 16), ("idenv", 16)):
        vecs[nm] = nc.dram_tensor(nm, [dim], f32, kind="ExternalInput")
    f1w_ext = nc.dram_tensor("f1wT", [1024, 512], f32, kind="ExternalInput")
    f2w_ext = nc.dram_tensor("f2wT", [512, 256], f32, kind="ExternalInput")
    f3w_ext = nc.dram_tensor("f3wT", [256, 16], f32, kind="ExternalInput")
    out_ext = nc.dram_tensor("out", [B, 4, 4], f32, kind="ExternalOutput")
    dbgx_ext = nc.dram_tensor("dbgx", [64, NH], f32, kind="ExternalOutput")
    dbgst_ext = nc.dram_tensor("dbgst", [2, 64], f32, kind="ExternalOutput")
    dbgidx_ext = nc.dram_tensor("dbgidx", [128, NT, 40], i32, kind="ExternalOutput")
    dbgm_ext = nc.dram_tensor("dbgm", [128, NT, 64], f32, kind="ExternalOutput")

    uT_dram = [nc.dram_tensor(f"uT{li}", [N, co], f32) for li, (_, co) in enumerate(BLOCKS)]
    st_in = [nc.dram_tensor(f"stin{li}", [2, co], f32) for li, (_, co) in enumerate(BLOCKS)]
    st_out = [nc.dram_tensor(f"stout{li}", [2, co], f32, addr_space="Shared")
              for li, (_, co) in enumerate(BLOCKS)]
    xh_in = [nc.dram_tensor(f"xhin{li}", [co, NH], f32) for li, (_, co) in enumerate(BLOCKS)]
    xh_out = [nc.dram_tensor(f"xhout{li}", [2, co, NH], f32)
              for li, (_, co) in enumerate(BLOCKS)]
    st5_in = nc.dram_tensor("st5in", [2, 1024], f32)
    st5_out = nc.dram_tensor("st5out", [2, 1024], f32, addr_space="Shared")
    fin_in = nc.dram_tensor("finin", [3, 1024], f32)
    fin_out = nc.dram_tensor("finout", [8, 3, 1024], f32, addr_space="Shared")

    with TileContext(nc) as tc:
        nc.partition_id()
        with (tc.tile_pool(name="consts", bufs=1) as consts,
              tc.tile_pool(name="xpool", bufs=2) as xpool,
              tc.tile_pool(name="dpool", bufs=1) as dpool,
              tc.tile_pool(name="gpool", bufs=2) as gpool,
              tc.tile_pool(name="spool", bufs=1) as spool,
              tc.tile_pool(name="blkpool", bufs=1) as blkpool,
              tc.tile_pool(name="psum", bufs=1, space="PSUM") as psum,
              tc.tile_pool(name="psum2", bufs=2, space="PSUM") as psum2,
              tc.tile_pool(name="psum_acc", bufs=1, space="PSUM") as psum_acc):

            ident = consts.tile([128, 128], f32)
            make_identity(nc, ident)
            ones1 = consts.tile([1, 128], f32)
            nc.vector.memset(ones1, 1.0)
            onescol = consts.tile([128, 1], f32)
            nc.vector.memset(onescol, 1.0)
            segoff = consts.tile([128, NCAND], i16)
            nc.gpsimd.iota(segoff, pattern=[[SL, SEG], [0, 8]], base=0, channel_multiplier=0)
            rank1 = consts.tile([128, 40], i16)
            nc.gpsimd.iota(rank1, pattern=[[1, 40]], base=1, channel_multiplier=0)

            Xc = xpool.tile([128, 1, N], f32, tag="X")
            nc.sync.dma_start(out=Xc[:3, 0, :], in_=x_ext[:, :])
            negs2 = blkpool.tile([1, N], f32, tag="negs2")
            xsq = dpool.tile([128, 1, N], f32, tag="D")
            nc.scalar.square(xsq[:3, 0, :], Xc[:3, 0, :])
            for jc in range(8):
                nps = psum.tile([1, 512], f32, tag="aux")
                nc.tensor.matmul(nps, onescol[:3, :1], xsq[:3, 0, jc * 512:(jc + 1) * 512],
                                 start=True, stop=True)
                nc.scalar.mul(negs2[:, jc * 512:(jc + 1) * 512], nps, -0.5)

            idxall = blkpool.tile([128, NT, 40], i32, tag="idxall")

            # ---------------- edge blocks ----------------
            for li, (CI, CO) in enumerate(BLOCKS):
                CT = (CI + 127) // 128
                OT = (CO + 127) // 128
                KP = min(CI, 128)
                wd = blkpool.tile([128, CT, CO], f32, tag="wd")
                wv = blkpool.tile([128, CT, CO], f32, tag="wv")
                nc.sync.dma_start(out=wd[:KP, :, :],
                                  in_=wdx[li].rearrange("(t p) o -> p t o", p=KP))
                nc.sync.dma_start(out=wv[:KP, :, :],
                                  in_=wvx[li].rearrange("(t p) o -> p t o", p=KP))
                gvec = _chan(nc, spool, ggx[li], OT, "gvec")
                evec = _chan(nc, spool, eex[li], OT, "evec")

                vT = blkpool.tile([128, NT, CO], f32, tag="vT")
                # uT/vT tables (all 32 j-tiles); uT -> DRAM
                sc_tab = nc.enter_named_scope(f"b{li}_tables", False)
                for jt in range(32):
                    uvp = psum.tile([128, 2 * CO], f32, tag="uvp")
                    for ct in range(CT):
                        kk = min(128, CI - ct * 128)
                        lhs = Xc[:kk, ct, jt * 128:(jt + 1) * 128]
                        nc.tensor.matmul(uvp[:, :CO], lhs, wd[:kk, ct, :],
                                         start=(ct == 0), stop=(ct == CT - 1))
                        nc.tensor.matmul(uvp[:, CO:], lhs, wv[:kk, ct, :],
                                         start=(ct == 0), stop=(ct == CT - 1))
                    ut_sb = spool.tile([128, CO], f32, tag="ut_sb")
                    nc.scalar.copy(ut_sb, uvp[:, :CO])
                    nc.sync.dma_start(out=uT_dram[li][jt * 128:(jt + 1) * 128, :], in_=ut_sb)
                    if jt < NT:
                        nc.vector.tensor_copy(vT[:, jt, :], uvp[:, CO:])
                # bias add to vT: replicate b row then add
                brow1 = spool.tile([1, CO], f32, tag="brow1")
                nc.sync.dma_start(out=brow1[:, :], in_=bbx[li][None, :])
                brow_sb = spool.tile([128, CO], f32, tag="brow_sb")
                for half in range((CO + 511) // 512):
                    w = min(512, CO - half * 512)
                    bps = psum.tile([128, 512], f32, tag="aux")
                    nc.tensor.matmul(bps[:, :w], ones1[:1, :], brow1[:, half * 512:half * 512 + w],
                                     start=True, stop=True)
                    nc.vector.tensor_copy(brow_sb[:, half * 512:half * 512 + w], bps[:, :w])
                for jt in range(NT):
                    nc.vector.tensor_add(vT[:, jt, :], vT[:, jt, :], brow_sb)
                nc.leave_named_scope(f"b{li}_tables", sc_tab[0], False)

                st_v = [psum_acc.tile([128, 4], f32, tag=f"stv{h}", name=f"stv{li}_{h}") for h in range(OT)]
                su_first = [True] * OT
                SQrun = spool.tile([128, CO], f32, tag="SQrun")
                CRrun = spool.tile([128, CO], f32, tag="CRrun")
                VVrun = spool.tile([128, CO], f32, tag="VVrun")
                S1r = spool.tile([128, CO], f32, tag="S1r")
                nc.vector.memset(SQrun, 0.0)
                nc.vector.memset(CRrun, 0.0)
                nc.vector.memset(VVrun, 0.0)
                Mall = blkpool.tile([128, NT, CO], f32, tag="Mall")

                nph = {64: 1, 128: 2, 256: 4}[CO]
                spp = 40 // nph
                total_qs = NT * 40

                for r in range(NT):
                    D = dpool.tile([128, N], f32, tag="D")
                    for jc in range(8):
                        dps = psum2.tile([128, 512], f32, tag="dps")
                        nc.tensor.matmul(dps, ones1[:1, :], negs2[:, jc * 512:(jc + 1) * 512],
                                         start=True, stop=False)
                        for ct in range(CT):
                            kk = min(128, CI - ct * 128)
                            nc.tensor.matmul(dps, Xc[:kk, ct, r * 128:(r + 1) * 128],
                                             Xc[:kk, ct, jc * 512:(jc + 1) * 512],
                                             start=False, stop=(ct == CT - 1))
                        nc.scalar.copy(D[:, jc * 512:(jc + 1) * 512], dps)
                    # ---- exact top-40 ----
                    candv = spool.tile([128, SEG, 8], f32, tag="candv")
                    candl = spool.tile([128, SEG, 8], u16, tag="candl")
                    for s in range(SEG):
                        nc.vector.max(out=candv[:, s], in_=D[:, s * SL:(s + 1) * SL])
                        nc.vector.max_index(out=candl[:, s], in_max=candv[:, s],
                                            in_values=D[:, s * SL:(s + 1) * SL])
                    cvf = candv.rearrange("p a b -> p (a b)")
                    candg = spool.tile([128, NCAND], i16, tag="candg")
                    nc.vector.tensor_tensor(out=candg,
                                            in0=candl.rearrange("p a b -> p (a b)").bitcast(i16),
                                            in1=segoff, op=OP.add)
                    work = spool.tile([128, NCAND], f32, tag="work")
                    nc.vector.tensor_copy(work, cvf)
                    pos = spool.tile([128, 40], u16, tag="pos")
                    top8 = spool.tile([128, 8], f32, tag="top8")
                    for rr in range(5):
                        nc.vector.max(out=top8, in_=work)
                        nc.vector.max_index(out=pos[:, rr * 8:(rr + 1) * 8], in_max=top8,
                                            in_values=cvf)
                        nc.vector.match_replace(out=work, in_to_replace=top8, in_values=work,
                                                imm_value=-1e30)
                    rankmap = spool.tile([128, NCAND], i16, tag="rankmap")
                    nc.gpsimd.local_scatter(out_ap=rankmap, data_ap=rank1,
                                            idxs_ap=pos.bitcast(i16), channels=128,
                                            num_elems=NCAND, num_idxs=40)
                    ridx = spool.tile([128, NCAND], i16, tag="ridx")
                    nc.vector.tensor_scalar(ridx, rankmap, 1, scalar2=None, op0=OP.subtract)
                    fidx = spool.tile([128, 40], i16, tag="fidx")
                    nc.gpsimd.local_scatter(out_ap=fidx, data_ap=candg, idxs_ap=ridx,
                                            channels=128, num_elems=40, num_idxs=NCAND)
                    nc.vector.tensor_copy(idxall[:, r, :], fidx)

                # ---- gather + max-reduce + stats (GPSIMD-bound pass) ----
                for r in range(NT):
                    for ph in range(nph):
                        G = gpool.tile([128, spp, CO], f32, tag="G")
                        for sloc in range(spp):
                            s = ph * spp + sloc
                            if s == 0:
                                nc.sync.dma_start(out=G[:, 0, :],
                                                  in_=uT_dram[li][r * 128:(r + 1) * 128, :])
                            else:
                                nc.gpsimd.indirect_dma_start(
                                    out=G[:, sloc, :], out_offset=None,
                                    in_=uT_dram[li][:, :],
                                    in_offset=bass.IndirectOffsetOnAxis(
                                        ap=idxall[:, r, s:s + 1], axis=0))
                        if ph == 0:
                            nc.vector.tensor_reduce(out=Mall[:, r, :],
                                                    in_=G.rearrange("p s c -> p c s"),
                                                    axis=AX.X, op=OP.max)
                        else:
                            mtmp = spool.tile([128, CO], f32, tag="mtmp")
                            nc.vector.tensor_reduce(out=mtmp, in_=G.rearrange("p s c -> p c s"),
                                                    axis=AX.X, op=OP.max)
                            nc.vector.tensor_max(Mall[:, r, :], Mall[:, r, :], mtmp)
                        for sloc in range(spp):
                            qs_i = r * 40 + ph * spp + sloc
                            last = (qs_i == total_qs - 1)
                            for h in range(OT):
                                mh = min(128, CO - h * 128)
                                lhs = G[:, sloc, h * 128:h * 128 + mh]
                                nc.tensor.matmul(st_v[h][:mh, 1:2], lhs, onescol,
                                                 start=su_first[h], stop=last)
                                su_first[h] = False
                        s1tmp = spool.tile([128, CO], f32, tag="qtmp", name="s1tmp")
                        nc.vector.tensor_reduce(out=s1tmp, in_=G.rearrange("p s c -> p c s"),
                                                axis=AX.X, op=OP.add)
                        if ph == 0:
                            nc.vector.tensor_copy(S1r, s1tmp)
                        else:
                            nc.vector.tensor_add(S1r, S1r, s1tmp)
                        # square G in place (all other readers are done), then reduce
                        nc.scalar.square(G.rearrange("p s c -> p (s c)"),
                                         G.rearrange("p s c -> p (s c)"))
                        sqtmp = spool.tile([128, CO], f32, tag="qtmp")
                        nc.vector.tensor_reduce(out=sqtmp, in_=G.rearrange("p s c -> p c s"),
                                                axis=AX.X, op=OP.add)
                        nc.vector.tensor_add(SQrun, SQrun, sqtmp)
                    vp_ = spool.tile([128, CO], f32, tag="qtmp", name="vp_")
                    nc.vector.tensor_mul(vp_, S1r, vT[:, r, :])
                    nc.vector.tensor_add(CRrun, CRrun, vp_)
                    nc.vector.tensor_mul(vp_, vT[:, r, :], vT[:, r, :])
                    nc.vector.tensor_add(VVrun, VVrun, vp_)

                # v sums
                sc_st = nc.enter_named_scope(f"b{li}_stats", False)
                for q in range(NT):
                    for h in range(OT):
                        mh = min(128, CO - h * 128)
                        lhs = vT[:, q, h * 128:h * 128 + mh]
                        nc.tensor.matmul(st_v[h][:mh, 0:1], lhs, onescol,
                                         start=(q == 0), stop=(q == NT - 1))
                # partition-reduce the quadratic runs
                qred = psum.tile([128, 3], f32, tag="aux")
                qred_sb = spool.tile([128, OT, 3], f32, tag="qred_sb")
                for h in range(OT):
                    mh = min(128, CO - h * 128)
                    nc.tensor.matmul(qred[:mh, 0:1], SQrun[:, h * 128:h * 128 + mh], onescol,
                                     start=True, stop=True)
                    nc.tensor.matmul(qred[:mh, 1:2], CRrun[:, h * 128:h * 128 + mh], onescol,
                                     start=True, stop=True)
                    nc.tensor.matmul(qred[:mh, 2:3], VVrun[:, h * 128:h * 128 + mh], onescol,
                                     start=True, stop=True)
                    nc.vector.tensor_copy(qred_sb[:, h, :], qred)
                # assemble stats per channel: [c-part, OT, 2]
                stats_sb = spool.tile([128, OT, 2], f32, tag="stats_sb")
                for h in range(OT):
                    s1 = spool.tile([128, 1], f32, tag="s1")
                    nc.vector.tensor_scalar(s1, st_v[h][:, 0:1], float(K), scalar2=None, op0=OP.mult)
                    nc.vector.tensor_add(stats_sb[:, h, 0:1], st_v[h][:, 1:2], s1)
                    nc.vector.tensor_scalar(s1, qred_sb[:, h, 2:3], float(K), scalar2=None, op0=OP.mult)
                    nc.vector.tensor_add(s1, s1, qred_sb[:, h, 0:1])
                    sc2 = spool.tile([128, 1], f32, tag="sc2")
                    nc.vector.tensor_scalar(sc2, qred_sb[:, h, 1:2], 2.0, scalar2=None, op0=OP.mult)
                    nc.vector.tensor_add(stats_sb[:, h, 1:2], s1, sc2)
                PP = min(CO, 128)
                for si in range(2):
                    nc.sync.dma_start(out=st_in[li][si].rearrange("(t p) -> p t", p=PP),
                                      in_=stats_sb[:PP, :, si])
                nc.gpsimd.collective_compute("AllReduce", OP.add, replica_groups=ALL8,
                                             ins=[st_in[li][:, :].opt()],
                                             outs=[st_out[li][:, :].opt()])
                stg = spool.tile([128, OT, 2], f32, tag="stg")
                for si in range(2):
                    nc.sync.dma_start(out=stg[:PP, :, si],
                                      in_=st_out[li][si].rearrange("(t p) -> p t", p=PP))
                CNT = float(B * N * K)
                mu = spool.tile([128, OT], f32, tag="mu")
                var = spool.tile([128, OT], f32, tag="var")
                nc.vector.tensor_scalar(mu, stg[:, :, 0], 1.0 / CNT, scalar2=None, op0=OP.mult)
                nc.vector.tensor_scalar(var, stg[:, :, 1], 1.0 / CNT, scalar2=None, op0=OP.mult)
                musq = spool.tile([128, OT], f32, tag="musq")
                nc.vector.tensor_mul(musq, mu, mu)
                nc.vector.tensor_sub(var, var, musq)
                nc.vector.tensor_scalar_add(var, var, EPS)
                nc.scalar.sqrt(var, var)
                rs = spool.tile([128, OT], f32, tag="rs")
                nc.vector.reciprocal(rs, var)
                scale = spool.tile([128, OT], f32, tag="scale")
                nc.vector.tensor_mul(scale, rs, gvec)
                sq2f = SQ2 if li < 3 else 1.0   # last block: no sqrt2 prescale downstream
                nc.vector.tensor_scalar(scale, scale, sq2f, scalar2=None, op0=OP.mult)
                toff = spool.tile([128, OT], f32, tag="toff")
                nc.vector.tensor_mul(toff, mu, scale)
                esc = spool.tile([128, OT], f32, tag="esc")
                nc.vector.tensor_scalar(esc, evec, sq2f, scalar2=None, op0=OP.mult)
                nc.vector.tensor_sub(toff, esc, toff)
                nc.leave_named_scope(f"b{li}_stats", sc_st[0], False)
                sc_ex = nc.enter_named_scope(f"b{li}_exch", False)

                nc.vector.tensor_add(Mall.rearrange("p q c -> p (q c)"),
                                     Mall.rearrange("p q c -> p (q c)"),
                                     vT.rearrange("p q c -> p (q c)"))
                Xn = xpool.tile([128, OT, N], f32, tag="X")
                for q in range(NT):
                    for h in range(OT):
                        mh = min(128, CO - h * 128)
                        tp = psum.tile([128, 128], f32, tag="aux")
                        nc.tensor.transpose(tp[:mh, :], Mall[:, q, h * 128:h * 128 + mh], ident)
                        zaf = spool.tile([128, 128], f32, tag="sqb", name="zaf")
                        nc.vector.tensor_scalar(zaf[:mh, :], tp[:mh, :],
                                                scale[:mh, h:h + 1], scalar2=toff[:mh, h:h + 1],
                                                op0=OP.mult, op1=OP.add)
                        nc.vector.scalar_tensor_tensor(Xn[:mh, h, q * 128:(q + 1) * 128],
                                                       in0=zaf[:mh, :], scalar=0.2,
                                                       in1=zaf[:mh, :],
                                                       op0=OP.mult, op1=OP.max)
                nc.sync.dma_start(out=xh_in[li].rearrange("(t p) n -> p t n", p=PP),
                                  in_=Xn[:PP, :, 0:NH])
                nc.gpsimd.collective_compute("AllGather", OP.bypass, replica_groups=PAIRS,
                                             ins=[xh_in[li][:, :].opt()],
                                             outs=[xh_out[li][:, :, :].opt()])

                pid = nc.gpsimd.partition_id()
                with tc.If(pid % 2 < 1) as cmp:
                    nc.gpsimd.dma_start(out=Xn[:PP, :, NH:N],
                                        in_=xh_out[li][1].rearrange("(t p) n -> p t n", p=PP))
                with cmp.Else():
                    nc.gpsimd.dma_start(out=Xn[:PP, :, NH:N],
                                        in_=xh_out[li][0].rearrange("(t p) n -> p t n", p=PP))
                nc.leave_named_scope(f"b{li}_exch", sc_ex[0], False)
                if li < 3:
                    xsq2 = dpool.tile([128, OT, N], f32, tag="D")
                    nc.scalar.square(xsq2.rearrange("p t n -> p (t n)"),
                                     Xn.rearrange("p t n -> p (t n)"))
                    for jc in range(8):
                        nps = psum.tile([1, 512], f32, tag="aux")
                        for h in range(OT):
                            mh = min(128, CO - h * 128)
                            nc.tensor.matmul(nps, onescol[:mh, :1],
                                             xsq2[:mh, h, jc * 512:(jc + 1) * 512],
                                             start=(h == 0), stop=(h == OT - 1))
                        nc.scalar.mul(negs2[:, jc * 512:(jc + 1) * 512], nps, -0.5)
                Xc = Xn

            # ---------------- tail ----------------
            sc_tl = nc.enter_named_scope("tail", False)
            cat = xpool.tile([128, 4, NH], f32, tag="X")
            nc.sync.dma_start(out=cat[0:64, 0, :], in_=xh_in[0][:, :])
            nc.sync.dma_start(out=cat[64:128, 0, :], in_=xh_in[1][:, :])
            nc.sync.dma_start(out=cat[:, 1, :], in_=xh_in[2][:, :])
            nc.sync.dma_start(out=cat[:, 2:4, :],
                              in_=xh_in[3].rearrange("(t p) n -> p t n", p=128))
            w5sb = dpool.tile([128, 4, 1024], f32, tag="D")
            nc.sync.dma_start(out=w5sb, in_=w5_ext.rearrange("(t p) o -> p t o", p=128))
            b5c = _chan(nc, spool, vecs["b5"], 8, "b5c")
            g5c = _chan(nc, spool, vecs["g5"], 8, "g5c")
            e5c = _chan(nc, spool, vecs["e5"], 8, "e5c")
            tbc = _chan(nc, spool, vecs["tb"], 8, "tbc")
            tgc = _chan(nc, spool, vecs["tg"], 8, "tgc")
            tec = _chan(nc, spool, vecs["te"], 8, "tec")

            st5 = spool.tile([128, 8, 2], f32, tag="st5")
            acc_s = spool.tile([128, 8, 4], f32, tag="accs")
            acc_q = spool.tile([128, 8, 4], f32, tag="accq")
            for nk in range(4):
                for mt in range(8):
                    hp = psum2.tile([128, 512], f32, tag="dps")
                    for ct in range(4):
                        nc.tensor.matmul(hp, w5sb[:, ct, mt * 128:(mt + 1) * 128],
                                         cat[:, ct, nk * 512:(nk + 1) * 512],
                                         start=(ct == 0), stop=(ct == 3))
                    hbuf = spool.tile([128, 512], f32, tag="hbuf")
                    nc.vector.tensor_scalar(hbuf, hp, b5c[:, mt:mt + 1], scalar2=None, op0=OP.add)
                    scr0 = spool.tile([128, 512], f32, tag="sqb")
                    nc.scalar.activation(scr0, hbuf, ACTF.Copy,
                                         accum_out=acc_s[:, mt, nk:nk + 1])
                    nc.scalar.activation(scr0, hbuf, ACTF.Square,
                                         accum_out=acc_q[:, mt, nk:nk + 1])
            nc.vector.tensor_reduce(out=st5[:, :, 0], in_=acc_s, axis=AX.X, op=OP.add)
            nc.vector.tensor_reduce(out=st5[:, :, 1], in_=acc_q, axis=AX.X, op=OP.add)
            for si in range(2):
                nc.sync.dma_start(out=st5_in[si].rearrange("(t p) -> p t", p=128),
                                  in_=st5[:, :, si])
            nc.gpsimd.collective_compute("AllReduce", OP.add, replica_groups=ALL8,
                                         ins=[st5_in[:, :].opt()], outs=[st5_out[:, :].opt()])
            st5g = spool.tile([128, 8, 2], f32, tag="st5g")
            for si in range(2):
                nc.sync.dma_start(out=st5g[:, :, si],
                                  in_=st5_out[si].rearrange("(t p) -> p t", p=128))

            def bn_coeffs(stat_tile, cnt, gt, et, tagp):
                mu_ = spool.tile([128, 8], f32, tag=f"mu{tagp}")
                va_ = spool.tile([128, 8], f32, tag=f"va{tagp}")
                nc.vector.tensor_scalar(mu_, stat_tile[:, :, 0], 1.0 / cnt, scalar2=None, op0=OP.mult)
                nc.vector.tensor_scalar(va_, stat_tile[:, :, 1], 1.0 / cnt, scalar2=None, op0=OP.mult)
                m2_ = spool.tile([128, 8], f32, tag=f"m2{tagp}")
                nc.vector.tensor_mul(m2_, mu_, mu_)
                nc.vector.tensor_sub(va_, va_, m2_)
                nc.vector.tensor_scalar_add(va_, va_, EPS)
                nc.scalar.sqrt(va_, va_)
                rs_ = spool.tile([128, 8], f32, tag=f"rs{tagp}")
                nc.vector.reciprocal(rs_, va_)
                sc_ = spool.tile([128, 8], f32, tag=f"sc{tagp}")
                nc.vector.tensor_mul(sc_, rs_, gt)
                to_ = spool.tile([128, 8], f32, tag=f"to{tagp}")
                nc.vector.tensor_mul(to_, mu_, sc_)
                nc.vector.tensor_sub(to_, et, to_)
                return sc_, to_

            sc5, t5 = bn_coeffs(st5g, float(B * N), g5c, e5c, "5")
            # t5' = t5 + sc5*b5 (pass B recomputes h5 without bias)
            t5p = spool.tile([128, 8], f32, tag="t5p")
            nc.vector.tensor_mul(t5p, sc5, b5c)
            nc.vector.tensor_add(t5p, t5p, t5)

            twsb = xpool.tile([128, 8, 1024], f32, tag="X")
            nc.sync.dma_start(out=twsb, in_=tw_ext.rearrange("(t p) o -> p t o", p=128))
            feat = gpool.tile([128, 8, 512], f32, tag="G")
            acc_ts = spool.tile([128, 8, 4], f32, tag="accs2")
            acc_tq = spool.tile([128, 8, 4], f32, tag="accq2")
            hmax = spool.tile([128, 8, 4], f32, tag="hmax")
            for nk in range(4):
                for mt in range(8):
                    hp = psum2.tile([128, 512], f32, tag="dps")
                    for ct in range(4):
                        nc.tensor.matmul(hp, w5sb[:, ct, mt * 128:(mt + 1) * 128],
                                         cat[:, ct, nk * 512:(nk + 1) * 512],
                                         start=(ct == 0), stop=(ct == 3))
                    zaf2 = spool.tile([128, 512], f32, tag="hbuf", name="zaf2")
                    nc.vector.tensor_scalar(zaf2, hp, sc5[:, mt:mt + 1],
                                            scalar2=t5p[:, mt:mt + 1], op0=OP.mult, op1=OP.add)
                    nc.vector.scalar_tensor_tensor(feat[:, mt, :], in0=zaf2, scalar=0.2,
                                                   in1=zaf2, op0=OP.mult, op1=OP.max)
                for mt in range(8):
                    hp2 = psum2.tile([128, 512], f32, tag="dps")
                    for ct in range(8):
                        nc.tensor.matmul(hp2, twsb[:, ct, mt * 128:(mt + 1) * 128],
                                         feat[:, ct, :],
                                         start=(ct == 0), stop=(ct == 7))
                    hbt = spool.tile([128, 512], f32, tag="hbuf")
                    nc.vector.tensor_scalar(hbt, hp2, tbc[:, mt:mt + 1], scalar2=None, op0=OP.add)
                    scr1 = spool.tile([128, 512], f32, tag="sqb")
                    nc.scalar.activation(scr1, hbt, ACTF.Copy,
                                         accum_out=acc_ts[:, mt, nk:nk + 1])
                    nc.scalar.activation(scr1, hbt, ACTF.Square,
                                         accum_out=acc_tq[:, mt, nk:nk + 1])
                    nc.vector.tensor_reduce(out=hmax[:, mt, nk:nk + 1], in_=hbt,
                                            axis=AX.X, op=OP.max)
            fin_sb = spool.tile([128, 8, 3], f32, tag="fin_sb")
            nc.vector.tensor_reduce(out=fin_sb[:, :, 0], in_=acc_ts, axis=AX.X, op=OP.add)
            nc.vector.tensor_reduce(out=fin_sb[:, :, 1], in_=acc_tq, axis=AX.X, op=OP.add)
            nc.vector.tensor_reduce(out=fin_sb[:, :, 2], in_=hmax, axis=AX.X, op=OP.max)
            for si in range(3):
                nc.sync.dma_start(out=fin_in[si].rearrange("(t p) -> p t", p=128),
                                  in_=fin_sb[:, :, si])
            nc.gpsimd.collective_compute("AllGather", OP.bypass, replica_groups=ALL8,
                                         ins=[fin_in[:, :].opt()],
                                         outs=[fin_out[:, :, :].opt()])
            allf = spool.tile([128, 8, 8, 3], f32, tag="allf")
            for ri in range(8):
                for si in range(3):
                    nc.sync.dma_start(out=allf[:, ri, :, si],
                                      in_=fin_out[ri, si].rearrange("(t p) -> p t", p=128))
            tws = spool.tile([128, 8, 2], f32, tag="tws")
            for si in range(2):
                nc.vector.tensor_reduce(out=tws[:, :, si],
                                        in_=allf[:, :, :, si].rearrange("p r t -> p t r"),
                                        axis=AX.X, op=OP.add)
            sct, tt = bn_coeffs(tws, float(B * N), tgc, tec, "t")
            P = spool.tile([128, 8, B], f32, tag="P")
            for b in range(B):
                pm = spool.tile([128, 8], f32, tag="pm")
                nc.vector.tensor_max(pm, allf[:, 2 * b, :, 2], allf[:, 2 * b + 1, :, 2])
                for mt in range(8):
                    nc.scalar.activation(P[:, mt, b:b + 1], pm[:, mt:mt + 1], ACTF.Relu,
                                         bias=tt[:, mt:mt + 1], scale=sct[:, mt:mt + 1])

            # ---------------- head ----------------
            def head_layer(Pin, tiles_in, cout, wext, bext, gk, ek, relu=True):
                ot = (cout + 127) // 128
                wsb = spool.tile([128, tiles_in, cout], f32, tag=f"hw{cout}")
                nc.sync.dma_start(out=wsb[:, :, :], in_=wext.rearrange("(t p) o -> p t o", p=128))
                hps = psum.tile([128, max(B * ot, B)], f32, tag="aux")
                for h in range(ot):
                    mh = min(128, cout - h * 128)
                    for ct in range(tiles_in):
                        nc.tensor.matmul(hps[:mh, h * B:(h + 1) * B],
                                         wsb[:, ct, h * 128:h * 128 + mh], Pin[:, ct, :],
                                         start=(ct == 0), stop=(ct == tiles_in - 1))
                bc = spool.tile([128, ot], f32, tag=f"hb{cout}")
                if cout >= 128:
                    nc.sync.dma_start(out=bc[:, :], in_=bext.rearrange("(t p) -> p t", p=128))
                else:
                    nc.sync.dma_start(out=bc[:cout, :], in_=bext[:, None])
                hout = spool.tile([128, ot, B], f32, tag=f"ho{cout}")
                for h in range(ot):
                    mh = min(128, cout - h * 128)
                    nc.vector.tensor_scalar(hout[:mh, h, :], hps[:mh, h * B:(h + 1) * B],
                                            bc[:mh, h:h + 1], scalar2=None, op0=OP.add)
                if gk is None:
                    return hout
                mub = spool.tile([128, ot], f32, tag=f"mb{cout}")
                nc.vector.tensor_reduce(out=mub, in_=hout, axis=AX.X, op=OP.add)
                nc.vector.tensor_scalar(mub, mub, 1.0 / B, scalar2=None, op0=OP.mult)
                hsq = spool.tile([128, ot, B], f32, tag=f"hs{cout}")
                nc.scalar.square(hsq.rearrange("p t b -> p (t b)"),
                                 hout.rearrange("p t b -> p (t b)"))
                vb = spool.tile([128, ot], f32, tag=f"vb{cout}")
                nc.vector.tensor_reduce(out=vb, in_=hsq, axis=AX.X, op=OP.add)
                nc.vector.tensor_scalar(vb, vb, 1.0 / B, scalar2=None, op0=OP.mult)
                mbs = spool.tile([128, ot], f32, tag=f"m2b{cout}")
                nc.vector.tensor_mul(mbs, mub, mub)
                nc.vector.tensor_sub(vb, vb, mbs)
                nc.vector.tensor_scalar_add(vb, vb, EPS)
                nc.scalar.sqrt(vb, vb)
                rsb = spool.tile([128, ot], f32, tag=f"rb{cout}")
                nc.vector.reciprocal(rsb, vb)
                gc = _chan(nc, spool, gk, ot, f"gc{cout}") if cout >= 128 else None
                ec = _chan(nc, spool, ek, ot, f"ec{cout}") if cout >= 128 else None
                scb = spool.tile([128, ot], f32, tag=f"scb{cout}")
                nc.vector.tensor_mul(scb, rsb, gc)
                tb_ = spool.tile([128, ot], f32, tag=f"tbb{cout}")
                nc.vector.tensor_mul(tb_, mub, scb)
                nc.vector.tensor_sub(tb_, ec, tb_)
                ho2 = spool.tile([128, ot, B], f32, tag=f"ho2{cout}")
                for h in range(ot):
                    mh = min(128, cout - h * 128)
                    nc.scalar.activation(ho2[:mh, h, :], hout[:mh, h, :], ACTF.Relu,
                                         bias=tb_[:mh, h:h + 1], scale=scb[:mh, h:h + 1])
                return ho2

            h1 = head_layer(P, 8, 512, f1w_ext, vecs["f1b"], vecs["f1g"], vecs["f1e"])
            h2 = head_layer(h1, 4, 256, f2w_ext, vecs["f2b"], vecs["f2g"], vecs["f2e"])
            h3 = head_layer(h2, 2, 16, f3w_ext, vecs["f3b"], None, None)
            idn = spool.tile([128, 1], f32, tag="idn")
            nc.sync.dma_start(out=idn[:16, :], in_=vecs["idenv"][:, None])
            h3f = spool.tile([128, B], f32, tag="h3f")
            nc.vector.tensor_add(h3f[:12, :], h3[:12, 0, :], idn[:12, :].to_broadcast([12, B]))
            approw = spool.tile([1, 4], f32, tag="approw")
            nc.vector.memset(approw, 0.0)
            nc.vector.memset(approw[:, 3:4], 1.0)
            for b in range(B):
                nc.sync.dma_start(out=out_ext[b, 0:3, :], in_=h3f[:12, b:b + 1])
                nc.sync.dma_start(out=out_ext[b, 3:4, :], in_=approw)
            nc.leave_named_scope("tail", sc_tl[0], False)
    nc.compile()
    return nc


_NC_CACHE = None


def kernel(**inputs):
    global _NC_CACHE
    from concourse.bass_utils import run_bass_kernel_spmd
    x = np.asarray(inputs['x'], np.float32)
    in_maps = []
    for core in range(8):
        b, h = core // 2, core % 2
        xr = np.roll(x[b], -h * NH, axis=1) * SQ2
        m = {"x0": np.ascontiguousarray(xr)}
        for li in range(4):
            W = np.asarray(inputs[f'w{li + 1}'], np.float32)
            C = W.shape[1] // 2
            Wd, Wv = W[:, :C], W[:, C:] - W[:, :C]
            # block inputs are sqrt2-scaled except block 4 output (li==3 input scaled too)
            m[f"wd{li}"] = np.ascontiguousarray(Wd.T / SQ2)
            m[f"wv{li}"] = np.ascontiguousarray(Wv.T / SQ2)
            m[f"bb{li}"] = np.asarray(inputs[f'b{li + 1}'], np.float32)
            m[f"gg{li}"] = np.asarray(inputs[f'g{li + 1}'], np.float32)
            m[f"ee{li}"] = np.asarray(inputs[f'e{li + 1}'], np.float32)
        w5T = np.asarray(inputs['w5'], np.float32).T.copy()
        w5T[:256, :] /= SQ2
        m["w5T"] = np.ascontiguousarray(w5T)
        m["twT"] = np.ascontiguousarray(np.asarray(inputs['tw'], np.float32).T)
        for nm in ("b5", "g5", "e5", "tb", "tg", "te", "f1b", "f1g", "f1e",
                   "f2b", "f2g", "f2e"):
            m[nm] = np.asarray(inputs[nm], np.float32)
        f3bp = np.zeros(16, np.float32); f3bp[:12] = np.asarray(inputs['f3b'], np.float32)
        m["f3b"] = f3bp
        idv = np.zeros(16, np.float32); idv[[0, 5, 10]] = 1.0
        m["idenv"] = idv
        m["f1wT"] = np.ascontiguousarray(np.asarray(inputs['f1w'], np.float32).T)
        m["f2wT"] = np.ascontiguousarray(np.asarray(inputs['f2w'], np.float32).T)
        f3T = np.asarray(inputs['f3w'], np.float32).T  # [256, 12]
        f3Tp = np.zeros((256, 16), np.float32)
        f3Tp[:, :12] = f3T
        m["f3wT"] = f3Tp
        in_maps.append(m)
    if _NC_CACHE is None:
        _NC_CACHE = build()
    kernel.last_in_maps = in_maps
    import os
    trace = bool(os.environ.get("KERNEL_TRACE"))
    res = run_bass_kernel_spmd(_NC_CACHE, in_maps, core_ids=list(range(8)), trace=trace)
    kernel.last_result = res
    return res.results[0]["out"].reshape(B, 4, 4)


# revision 33
# speedup vs baseline: 1.0105x; 1.0105x over previous
"""DGCNN transform-net forward on 8 TRN2 NeuronCores (Bass/Tile).

Sharding: cores 2b, 2b+1 handle cloud b; each computes knn/features for half
the cloud (local rows 0..2047 in a rotated frame). Pair AllGather per edge
block exchanges feature halves; 8-way AllReduce combines BN statistics.

Per edge block: h_ij = u_j + v_i with u = Wd x, v = (Wc-Wd) x + b.
Exact top-40 by distance via 32-segment top-8 + candidate rounds (DVE),
per-slot indirect-DMA gathers of u rows, DVE strided max-reduce, and
PE accumulation matmuls for the BN moments. max-over-k commutes with the
BN affine + leaky relu (positive scale), so only per-point maxima are kept.
"""
import sys
sys.path.insert(0, '/opt/trn_rl_repo')
import math
import numpy as np
import concourse.bass as bass
import concourse.mybir as mybir
from concourse import bacc
from concourse.tile import TileContext
from concourse.masks import make_identity

f32 = mybir.dt.float32
i16 = mybir.dt.int16
u16 = mybir.dt.uint16
i32 = mybir.dt.int32
AX = mybir.AxisListType
OP = mybir.AluOpType
ACTF = mybir.ActivationFunctionType

B, N, K = 4, 4096, 40
NH = N // 2
NT = NH // 128
SEG, SL = 32, 128
NCAND = SEG * 8
BLOCKS = [(3, 64), (64, 64), (64, 128), (128, 256)]
EPS = 1e-5
PAIRS = [[0, 1], [2, 3], [4, 5], [6, 7]]
ALL8 = [list(range(8))]
SQ2 = math.sqrt(2.0)


def _chan(nc, pool, ext, tiles, tag):
    t = pool.tile([128, tiles], f32, tag=tag)
    dim = ext.shape[0]
    if dim >= 128:
        nc.sync.dma_start(out=t[:, :], in_=ext.rearrange("(t p) -> p t", p=128))
    else:
        nc.sync.dma_start(out=t[:dim, 0:1], in_=ext[:, None])
    return t


def build():
    nc = bacc.Bacc("TRN2", debug=False)

    x_ext = nc.dram_tensor("x0", [3, N], f32, kind="ExternalInput")
    wdx, wvx, bbx, ggx, eex = [], [], [], [], []
    for li, (ci, co) in enumerate(BLOCKS):
        wdx.append(nc.dram_tensor(f"wd{li}", [ci, co], f32, kind="ExternalInput"))
        wvx.append(nc.dram_tensor(f"wv{li}", [ci, co], f32, kind="ExternalInput"))
        bbx.append(nc.dram_tensor(f"bb{li}", [co], f32, kind="ExternalInput"))
        ggx.append(nc.dram_tensor(f"gg{li}", [co], f32, kind="ExternalInput"))
        eex.append(nc.dram_tensor(f"ee{li}", [co], f32, kind="ExternalInput"))
    w5_ext = nc.dram_tensor("w5T", [512, 1024], f32, kind="ExternalInput")
    tw_ext = nc.dram_tensor("twT", [1024, 1024], f32, kind="ExternalInput")
    vecs = {}
    for nm, dim in (("b5", 1024), ("g5", 1024), ("e5", 1024), ("tb", 1024),
                    ("tg", 1024), ("te", 1024), ("f1b", 512), ("f1g", 512),
                    ("f1e", 512), ("f2b", 256), ("f2g", 256), ("f2e", 256),
                    ("f3b", 16), ("idenv", 16)):
        vecs[nm] = nc.dram_tensor(nm, [dim], f32, kind="ExternalInput")
    f1w_ext = nc.dram_tensor("f1wT", [1024, 512], f32, kind="ExternalInput")
    f2w_ext = nc.dram_tensor("f2wT", [512, 256], f32, kind="ExternalInput")
    f3w_ext = nc.dram_tensor("f3wT", [256, 16], f32, kind="ExternalInput")
    out_ext = nc.dram_tensor("out", [B, 4, 4], f32, kind="ExternalOutput")
    dbgx_ext = nc.dram_tensor("dbgx", [64, NH], f32, kind="ExternalOutput")
    dbgst_ext = nc.dram_tensor("dbgst", [2, 64], f32, kind="ExternalOutput")
    dbgidx_ext = nc.dram_tensor("dbgidx", [128, NT, 40], i32, kind="ExternalOutput")
    dbgm_ext = nc.dram_tensor("dbgm", [128, NT, 64], f32, kind="ExternalOutput")

    uT_dram = [nc.dram_tensor(f"uT{li}", [N, co], f32) for li, (_, co) in enumerate(BLOCKS)]
    st_in = [nc.dram_tensor(f"stin{li}", [2, co], f32) for li, (_, co) in enumerate(BLOCKS)]
    st_out = [nc.dram_tensor(f"stout{li}", [2, co], f32, addr_space="Shared")
              for li, (_, co) in enumerate(BLOCKS)]
    xh_in = [nc.dram_tensor(f"xhin{li}", [co, NH], f32) for li, (_, co) in enumerate(BLOCKS)]
    xh_out = [nc.dram_tensor(f"xhout{li}", [2, co, NH], f32)
              for li, (_, co) in enumerate(BLOCKS)]
    st5_in = nc.dram_tensor("st5in", [2, 1024], f32)
    st5_out = nc.dram_tensor("st5out", [2, 1024], f32, addr_space="Shared")
    fin_in = nc.dram_tensor("finin", [3, 1024], f32)
    fin_out = nc.dram_tensor("finout", [8, 3, 1024], f32, addr_space="Shared")

    with TileContext(nc) as tc:
        nc.partition_id()
        with (tc.tile_pool(name="consts", bufs=1) as consts,
              tc.tile_pool(name="xpool", bufs=2) as xpool,
              tc.tile_pool(name="dpool", bufs=1) as dpool,
              tc.tile_pool(name="gpool", bufs=2) as gpool,
              tc.tile_pool(name="spool", bufs=1) as spool,
              tc.tile_pool(name="blkpool", bufs=1) as blkpool,
              tc.tile_pool(name="psum", bufs=1, space="PSUM") as psum,
              tc.tile_pool(name="psum2", bufs=2, space="PSUM") as psum2,
              tc.tile_pool(name="psum_acc", bufs=1, space="PSUM") as psum_acc):

            ident = consts.tile([128, 128], f32)
            make_identity(nc, ident)
            ones1 = consts.tile([1, 128], f32)
            nc.vector.memset(ones1, 1.0)
            onescol = consts.tile([128, 1], f32)
            nc.vector.memset(onescol, 1.0)
            segoff = consts.tile([128, NCAND], i16)
            nc.gpsimd.iota(segoff, pattern=[[SL, SEG], [0, 8]], base=0, channel_multiplier=0)
            rank1 = consts.tile([128, 40], i16)
            nc.gpsimd.iota(rank1, pattern=[[1, 40]], base=1, channel_multiplier=0)

            Xc = xpool.tile([128, 1, N], f32, tag="X")
            nc.sync.dma_start(out=Xc[:3, 0, :], in_=x_ext[:, :])
            negs2 = blkpool.tile([1, N], f32, tag="negs2")
            xsq = dpool.tile([128, 1, N], f32, tag="D")
            nc.scalar.square(xsq[:3, 0, :], Xc[:3, 0, :])
            for jc in range(8):
                nps = psum.tile([1, 512], f32, tag="aux")
                nc.tensor.matmul(nps, onescol[:3, :1], xsq[:3, 0, jc * 512:(jc + 1) * 512],
                                 start=True, stop=True)
                nc.scalar.mul(negs2[:, jc * 512:(jc + 1) * 512], nps, -0.5)

            idxall = blkpool.tile([128, NT, 40], i32, tag="idxall")

            # ---------------- edge blocks ----------------
            for li, (CI, CO) in enumerate(BLOCKS):
                CT = (CI + 127) // 128
                OT = (CO + 127) // 128
                KP = min(CI, 128)
                wd = blkpool.tile([128, CT, CO], f32, tag="wd")
                wv = blkpool.tile([128, CT, CO], f32, tag="wv")
                nc.sync.dma_start(out=wd[:KP, :, :],
                                  in_=wdx[li].rearrange("(t p) o -> p t o", p=KP))
                nc.sync.dma_start(out=wv[:KP, :, :],
                                  in_=wvx[li].rearrange("(t p) o -> p t o", p=KP))
                gvec = _chan(nc, spool, ggx[li], OT, "gvec")
                evec = _chan(nc, spool, eex[li], OT, "evec")

                vT = blkpool.tile([128, NT, CO], f32, tag="vT")
                # uT/vT tables (all 32 j-tiles); uT -> DRAM
                sc_tab = nc.enter_named_scope(f"b{li}_tables", False)
                for jt in range(32):
                    uvp = psum.tile([128, 2 * CO], f32, tag="uvp")
                    for ct in range(CT):
                        kk = min(128, CI - ct * 128)
                        lhs = Xc[:kk, ct, jt * 128:(jt + 1) * 128]
                        nc.tensor.matmul(uvp[:, :CO], lhs, wd[:kk, ct, :],
                                         start=(ct == 0), stop=(ct == CT - 1))
                        nc.tensor.matmul(uvp[:, CO:], lhs, wv[:kk, ct, :],
                                         start=(ct == 0), stop=(ct == CT - 1))
                    ut_sb = spool.tile([128, CO], f32, tag="ut_sb")
                    nc.scalar.copy(ut_sb, uvp[:, :CO])
                    nc.sync.dma_start(out=uT_dram[li][jt * 128:(jt + 1) * 128, :], in_=ut_sb)
                    if jt < NT:
                        nc.vector.tensor_copy(vT[:, jt, :], uvp[:, CO:])
                # bias add to vT: replicate b row then add
                brow1 = spool.tile([1, CO], f32, tag="brow1")
                nc.sync.dma_start(out=brow1[:, :], in_=bbx[li][None, :])
                brow_sb = spool.tile([128, CO], f32, tag="brow_sb")
                for half in range((CO + 511) // 512):
                    w = min(512, CO - half * 512)
                    bps = psum.tile([128, 512], f32, tag="aux")
                    nc.tensor.matmul(bps[:, :w], ones1[:1, :], brow1[:, half * 512:half * 512 + w],
                                     start=True, stop=True)
                    nc.vector.tensor_copy(brow_sb[:, half * 512:half * 512 + w], bps[:, :w])
                for jt in range(NT):
                    nc.vector.tensor_add(vT[:, jt, :], vT[:, jt, :], brow_sb)
                nc.leave_named_scope(f"b{li}_tables", sc_tab[0], False)

                st_v = [psum_acc.tile([128, 4], f32, tag=f"stv{h}", name=f"stv{li}_{h}") for h in range(OT)]
                su_first = [True] * OT
                SQrun = spool.tile([128, CO], f32, tag="SQrun")
                CRrun = spool.tile([128, CO], f32, tag="CRrun")
                VVrun = spool.tile([128, CO], f32, tag="VVrun")
                S1r = spool.tile([128, CO], f32, tag="S1r")
                nc.vector.memset(SQrun, 0.0)
                nc.vector.memset(CRrun, 0.0)
                nc.vector.memset(VVrun, 0.0)
                Mall = blkpool.tile([128, NT, CO], f32, tag="Mall")

                nph = {64: 1, 128: 2, 256: 4}[CO]
                spp = 40 // nph
                total_qs = NT * 40

                for r in range(NT):
                    D = dpool.tile([128, N], f32, tag="D")
                    for jc in range(8):
                        dps = psum2.tile([128, 512], f32, tag="dps")
                        nc.tensor.matmul(dps, ones1[:1, :], negs2[:, jc * 512:(jc + 1) * 512],
                                         start=True, stop=False)
                        for ct in range(CT):
                            kk = min(128, CI - ct * 128)
                            nc.tensor.matmul(dps, Xc[:kk, ct, r * 128:(r + 1) * 128],
                                             Xc[:kk, ct, jc * 512:(jc + 1) * 512],
                                             start=False, stop=(ct == CT - 1))
                        nc.scalar.copy(D[:, jc * 512:(jc + 1) * 512], dps)
                    # ---- exact top-40 ----
                    candv = spool.tile([128, SEG, 8], f32, tag="candv")
                    candl = spool.tile([128, SEG, 8], u16, tag="candl")
                    for s in range(SEG):
                        nc.vector.max(out=candv[:, s], in_=D[:, s * SL:(s + 1) * SL])
                        nc.vector.max_index(out=candl[:, s], in_max=candv[:, s],
                                            in_values=D[:, s * SL:(s + 1) * SL])
                    cvf = candv.rearrange("p a b -> p (a b)")
                    candg = spool.tile([128, NCAND], i16, tag="candg")
                    nc.vector.tensor_tensor(out=candg,
                                            in0=candl.rearrange("p a b -> p (a b)").bitcast(i16),
                                            in1=segoff, op=OP.add)
                    work = spool.tile([128, NCAND], f32, tag="work")
                    nc.vector.tensor_copy(work, cvf)
                    pos = spool.tile([128, 40], u16, tag="pos", bufs=2)
                    top8 = spool.tile([128, 8], f32, tag="top8")
                    for rr in range(5):
                        nc.vector.max(out=top8, in_=work)
                        nc.vector.max_index(out=pos[:, rr * 8:(rr + 1) * 8], in_max=top8,
                                            in_values=cvf)
                        if rr < 4:
                            nc.vector.match_replace(out=work, in_to_replace=top8,
                                                    in_values=work, imm_value=-1e30)
                    rankmap = spool.tile([128, NCAND], i16, tag="rankmap")
                    nc.gpsimd.local_scatter(out_ap=rankmap, data_ap=rank1,
                                            idxs_ap=pos.bitcast(i16), channels=128,
                                            num_elems=NCAND, num_idxs=40)
                    ridx = spool.tile([128, NCAND], i16, tag="ridx")
                    nc.vector.tensor_scalar(ridx, rankmap, 1, scalar2=None, op0=OP.subtract)
                    fidx = spool.tile([128, 40], i16, tag="fidx", bufs=2)
                    nc.gpsimd.local_scatter(out_ap=fidx, data_ap=candg, idxs_ap=ridx,
                                            channels=128, num_elems=40, num_idxs=NCAND)
                    nc.vector.tensor_copy(idxall[:, r, :], fidx)

                # ---- gather + max-reduce + stats (GPSIMD-bound pass) ----
                for r in range(NT):
                    for ph in range(nph):
                        G = gpool.tile([128, spp, CO], f32, tag="G")
                        for sloc in range(spp):
                            s = ph * spp + sloc
                            if s == 0:
                                nc.sync.dma_start(out=G[:, 0, :],
                                                  in_=uT_dram[li][r * 128:(r + 1) * 128, :])
                            else:
                                nc.gpsimd.indirect_dma_start(
                                    out=G[:, sloc, :], out_offset=None,
                                    in_=uT_dram[li][:, :],
                                    in_offset=bass.IndirectOffsetOnAxis(
                                        ap=idxall[:, r, s:s + 1], axis=0))
                        if ph == 0:
                            nc.vector.tensor_reduce(out=Mall[:, r, :],
                                                    in_=G.rearrange("p s c -> p c s"),
                                                    axis=AX.X, op=OP.max)
                        else:
                            mtmp = spool.tile([128, CO], f32, tag="mtmp")
                            nc.vector.tensor_reduce(out=mtmp, in_=G.rearrange("p s c -> p c s"),
                                                    axis=AX.X, op=OP.max)
                            nc.vector.tensor_max(Mall[:, r, :], Mall[:, r, :], mtmp)
                        for sloc in range(spp):
                            qs_i = r * 40 + ph * spp + sloc
                            last = (qs_i == total_qs - 1)
                            for h in range(OT):
                                mh = min(128, CO - h * 128)
                                lhs = G[:, sloc, h * 128:h * 128 + mh]
                                nc.tensor.matmul(st_v[h][:mh, 1:2], lhs, onescol,
                                                 start=su_first[h], stop=last)
                                su_first[h] = False
                        s1tmp = spool.tile([128, CO], f32, tag="qtmp", name="s1tmp")
                        nc.vector.tensor_reduce(out=s1tmp, in_=G.rearrange("p s c -> p c s"),
                                                axis=AX.X, op=OP.add)
                        if ph == 0:
                            nc.vector.tensor_copy(S1r, s1tmp)
                        else:
                            nc.vector.tensor_add(S1r, S1r, s1tmp)
                        # square G in place (all other readers are done), then reduce
                        nc.scalar.square(G.rearrange("p s c -> p (s c)"),
                                         G.rearrange("p s c -> p (s c)"))
                        sqtmp = spool.tile([128, CO], f32, tag="qtmp")
                        nc.vector.tensor_reduce(out=sqtmp, in_=G.rearrange("p s c -> p c s"),
                                                axis=AX.X, op=OP.add)
                        nc.vector.tensor_add(SQrun, SQrun, sqtmp)
                    vp_ = spool.tile([128, CO], f32, tag="qtmp", name="vp_")
                    nc.vector.tensor_mul(vp_, S1r, vT[:, r, :])
                    nc.vector.tensor_add(CRrun, CRrun, vp_)
                    nc.vector.tensor_mul(vp_, vT[:, r, :], vT[:, r, :])
                    nc.vector.tensor_add(VVrun, VVrun, vp_)

                # v sums
                sc_st = nc.enter_named_scope(f"b{li}_stats", False)
                for q in range(NT):
                    for h in range(OT):
                        mh = min(128, CO - h * 128)
                        lhs = vT[:, q, h * 128:h * 128 + mh]
                        nc.tensor.matmul(st_v[h][:mh, 0:1], lhs, onescol,
                                         start=(q == 0), stop=(q == NT - 1))
                # partition-reduce the quadratic runs
                qred = psum.tile([128, 3], f32, tag="aux")
                qred_sb = spool.tile([128, OT, 3], f32, tag="qred_sb")
                for h in range(OT):
                    mh = min(128, CO - h * 128)
                    nc.tensor.matmul(qred[:mh, 0:1], SQrun[:, h * 128:h * 128 + mh], onescol,
                                     start=True, stop=True)
                    nc.tensor.matmul(qred[:mh, 1:2], CRrun[:, h * 128:h * 128 + mh], onescol,
                                     start=True, stop=True)
                    nc.tensor.matmul(qred[:mh, 2:3], VVrun[:, h * 128:h * 128 + mh], onescol,
                                     start=True, stop=True)
                    nc.vector.tensor_copy(qred_sb[:, h, :], qred)
                # assemble stats per channel: [c-part, OT, 2]
                stats_sb = spool.tile([128, OT, 2], f32, tag="stats_sb")
                for h in range(OT):
                    s1 = spool.tile([128, 1], f32, tag="s1")
                    nc.vector.tensor_scalar(s1, st_v[h][:, 0:1], float(K), scalar2=None, op0=OP.mult)
                    nc.vector.tensor_add(stats_sb[:, h, 0:1], st_v[h][:, 1:2], s1)
                    nc.vector.tensor_scalar(s1, qred_sb[:, h, 2:3], float(K), scalar2=None, op0=OP.mult)
                    nc.vector.tensor_add(s1, s1, qred_sb[:, h, 0:1])
                    sc2 = spool.tile([128, 1], f32, tag="sc2")
                    nc.vector.tensor_scalar(sc2, qred_sb[:, h, 1:2], 2.0, scalar2=None, op0=OP.mult)
                    nc.vector.tensor_add(stats_sb[:, h, 1:2], s1, sc2)
                PP = min(CO, 128)
                for si in range(2):
                    nc.sync.dma_start(out=st_in[li][si].rearrange("(t p) -> p t", p=PP),
                                      in_=stats_sb[:PP, :, si])
                nc.gpsimd.collective_compute("AllReduce", OP.add, replica_groups=ALL8,
                                             ins=[st_in[li][:, :].opt()],
                                             outs=[st_out[li][:, :].opt()])
                stg = spool.tile([128, OT, 2], f32, tag="stg")
                for si in range(2):
                    nc.sync.dma_start(out=stg[:PP, :, si],
                                      in_=st_out[li][si].rearrange("(t p) -> p t", p=PP))
                CNT = float(B * N * K)
                mu = spool.tile([128, OT], f32, tag="mu")
                var = spool.tile([128, OT], f32, tag="var")
                nc.vector.tensor_scalar(mu, stg[:, :, 0], 1.0 / CNT, scalar2=None, op0=OP.mult)
                nc.vector.tensor_scalar(var, stg[:, :, 1], 1.0 / CNT, scalar2=None, op0=OP.mult)
                musq = spool.tile([128, OT], f32, tag="musq")
                nc.vector.tensor_mul(musq, mu, mu)
                nc.vector.tensor_sub(var, var, musq)
                nc.vector.tensor_scalar_add(var, var, EPS)
                nc.scalar.sqrt(var, var)
                rs = spool.tile([128, OT], f32, tag="rs")
                nc.vector.reciprocal(rs, var)
                scale = spool.tile([128, OT], f32, tag="scale")
                nc.vector.tensor_mul(scale, rs, gvec)
                sq2f = SQ2 if li < 3 else 1.0   # last block: no sqrt2 prescale downstream
                nc.vector.tensor_scalar(scale, scale, sq2f, scalar2=None, op0=OP.mult)
                toff = spool.tile([128, OT], f32, tag="toff")
                nc.vector.tensor_mul(toff, mu, scale)
                esc = spool.tile([128, OT], f32, tag="esc")
                nc.vector.tensor_scalar(esc, evec, sq2f, scalar2=None, op0=OP.mult)
                nc.vector.tensor_sub(toff, esc, toff)
                nc.leave_named_scope(f"b{li}_stats", sc_st[0], False)
                sc_ex = nc.enter_named_scope(f"b{li}_exch", False)

                nc.vector.tensor_add(Mall.rearrange("p q c -> p (q c)"),
                                     Mall.rearrange("p q c -> p (q c)"),
                                     vT.rearrange("p q c -> p (q c)"))
                Xn = xpool.tile([128, OT, N], f32, tag="X")
                for q in range(NT):
                    for h in range(OT):
                        mh = min(128, CO - h * 128)
                        tp = psum.tile([128, 128], f32, tag="aux")
                        nc.tensor.transpose(tp[:mh, :], Mall[:, q, h * 128:h * 128 + mh], ident)
                        zaf = spool.tile([128, 128], f32, tag="sqb", name="zaf")
                        nc.vector.tensor_scalar(zaf[:mh, :], tp[:mh, :],
                                                scale[:mh, h:h + 1], scalar2=toff[:mh, h:h + 1],
                                                op0=OP.mult, op1=OP.add)
                        nc.vector.scalar_tensor_tensor(Xn[:mh, h, q * 128:(q + 1) * 128],
                                                       in0=zaf[:mh, :], scalar=0.2,
                                                       in1=zaf[:mh, :],
                                                       op0=OP.mult, op1=OP.max)
                nc.sync.dma_start(out=xh_in[li].rearrange("(t p) n -> p t n", p=PP),
                                  in_=Xn[:PP, :, 0:NH])
                nc.gpsimd.collective_compute("AllGather", OP.bypass, replica_groups=PAIRS,
                                             ins=[xh_in[li][:, :].opt()],
                                             outs=[xh_out[li][:, :, :].opt()])

                pid = nc.gpsimd.partition_id()
                with tc.If(pid % 2 < 1) as cmp:
                    nc.gpsimd.dma_start(out=Xn[:PP, :, NH:N],
                                        in_=xh_out[li][1].rearrange("(t p) n -> p t n", p=PP))
                with cmp.Else():
                    nc.gpsimd.dma_start(out=Xn[:PP, :, NH:N],
                                        in_=xh_out[li][0].rearrange("(t p) n -> p t n", p=PP))
                nc.leave_named_scope(f"b{li}_exch", sc_ex[0], False)
                if li < 3:
                    xsq2 = dpool.tile([128, OT, N], f32, tag="D")
                    nc.scalar.square(xsq2.rearrange("p t n -> p (t n)"),
                                     Xn.rearrange("p t n -> p (t n)"))
                    for jc in range(8):
                        nps = psum.tile([1, 512], f32, tag="aux")
                        for h in range(OT):
                            mh = min(128, CO - h * 128)
                            nc.tensor.matmul(nps, onescol[:mh, :1],
                                             xsq2[:mh, h, jc * 512:(jc + 1) * 512],
                                             start=(h == 0), stop=(h == OT - 1))
                        nc.scalar.mul(negs2[:, jc * 512:(jc + 1) * 512], nps, -0.5)
                Xc = Xn

            # ---------------- tail ----------------
            sc_tl = nc.enter_named_scope("tail", False)
            cat = xpool.tile([128, 4, NH], f32, tag="X")
            nc.sync.dma_start(out=cat[0:64, 0, :], in_=xh_in[0][:, :])
            nc.sync.dma_start(out=cat[64:128, 0, :], in_=xh_in[1][:, :])
            nc.sync.dma_start(out=cat[:, 1, :], in_=xh_in[2][:, :])
            nc.sync.dma_start(out=cat[:, 2:4, :],
                              in_=xh_in[3].rearrange("(t p) n -> p t n", p=128))
            w5sb = dpool.tile([128, 4, 1024], f32, tag="D")
            nc.sync.dma_start(out=w5sb, in_=w5_ext.rearrange("(t p) o -> p t o", p=128))
            b5c = _chan(nc, spool, vecs["b5"], 8, "b5c")
            g5c = _chan(nc, spool, vecs["g5"], 8, "g5c")
            e5c = _chan(nc, spool, vecs["e5"], 8, "e5c")
            tbc = _chan(nc, spool, vecs["tb"], 8, "tbc")
            tgc = _chan(nc, spool, vecs["tg"], 8, "tgc")
            tec = _chan(nc, spool, vecs["te"], 8, "tec")

            st5 = spool.tile([128, 8, 2], f32, tag="st5")
            acc_s = spool.tile([128, 8, 4], f32, tag="accs")
            acc_q = spool.tile([128, 8, 4], f32, tag="accq")
            for nk in range(4):
                for mt in range(8):
                    hp = psum2.tile([128, 512], f32, tag="dps")
                    for ct in range(4):
                        nc.tensor.matmul(hp, w5sb[:, ct, mt * 128:(mt + 1) * 128],
                                         cat[:, ct, nk * 512:(nk + 1) * 512],
                                         start=(ct == 0), stop=(ct == 3))
                    hbuf = spool.tile([128, 512], f32, tag="hbuf")
                    nc.vector.tensor_scalar(hbuf, hp, b5c[:, mt:mt + 1], scalar2=None, op0=OP.add)
                    scr0 = spool.tile([128, 512], f32, tag="sqb")
                    nc.scalar.activation(scr0, hbuf, ACTF.Copy,
                                         accum_out=acc_s[:, mt, nk:nk + 1])
                    nc.scalar.activation(scr0, hbuf, ACTF.Square,
                                         accum_out=acc_q[:, mt, nk:nk + 1])
            nc.vector.tensor_reduce(out=st5[:, :, 0], in_=acc_s, axis=AX.X, op=OP.add)
            nc.vector.tensor_reduce(out=st5[:, :, 1], in_=acc_q, axis=AX.X, op=OP.add)
            for si in range(2):
                nc.sync.dma_start(out=st5_in[si].rearrange("(t p) -> p t", p=128),
                                  in_=st5[:, :, si])
            nc.gpsimd.collective_compute("AllReduce", OP.add, replica_groups=ALL8,
                                         ins=[st5_in[:, :].opt()], outs=[st5_out[:, :].opt()])
            st5g = spool.tile([128, 8, 2], f32, tag="st5g")
            for si in range(2):
                nc.sync.dma_start(out=st5g[:, :, si],
                                  in_=st5_out[si].rearrange("(t p) -> p t", p=128))

            def bn_coeffs(stat_tile, cnt, gt, et, tagp):
                mu_ = spool.tile([128, 8], f32, tag=f"mu{tagp}")
                va_ = spool.tile([128, 8], f32, tag=f"va{tagp}")
                nc.vector.tensor_scalar(mu_, stat_tile[:, :, 0], 1.0 / cnt, scalar2=None, op0=OP.mult)
                nc.vector.tensor_scalar(va_, stat_tile[:, :, 1], 1.0 / cnt, scalar2=None, op0=OP.mult)
                m2_ = spool.tile([128, 8], f32, tag=f"m2{tagp}")
                nc.vector.tensor_mul(m2_, mu_, mu_)
                nc.vector.tensor_sub(va_, va_, m2_)
                nc.vector.tensor_scalar_add(va_, va_, EPS)
                nc.scalar.sqrt(va_, va_)
                rs_ = spool.tile([128, 8], f32, tag=f"rs{tagp}")
                nc.vector.reciprocal(rs_, va_)
                sc_ = spool.tile([128, 8], f32, tag=f"sc{tagp}")
                nc.vector.tensor_mul(sc_, rs_, gt)
                to_ = spool.tile([128, 8], f32, tag=f"to{tagp}")
                nc.vector.tensor_mul(to_, mu_, sc_)
                nc.vector.tensor_sub(to_, et, to_)
                return sc_, to_

            sc5, t5 = bn_coeffs(st5g, float(B * N), g5c, e5c, "5")
            # t5' = t5 + sc5*b5 (pass B recomputes h5 without bias)
            t5p = spool.tile([128, 8], f32, tag="t5p")
            nc.vector.tensor_mul(t5p, sc5, b5c)
            nc.vector.tensor_add(t5p, t5p, t5)

            twsb = xpool.tile([128, 8, 1024], f32, tag="X")
            nc.sync.dma_start(out=twsb, in_=tw_ext.rearrange("(t p) o -> p t o", p=128))
            feat = gpool.tile([128, 8, 512], f32, tag="G")
            acc_ts = spool.tile([128, 8, 4], f32, tag="accs2")
            acc_tq = spool.tile([128, 8, 4], f32, tag="accq2")
            hmax = spool.tile([128, 8, 4], f32, tag="hmax")
            for nk in range(4):
                for mt in range(8):
                    hp = psum2.tile([128, 512], f32, tag="dps")
                    for ct in range(4):
                        nc.tensor.matmul(hp, w5sb[:, ct, mt * 128:(mt + 1) * 128],
                                         cat[:, ct, nk * 512:(nk + 1) * 512],
                                         start=(ct == 0), stop=(ct == 3))
                    zaf2 = spool.tile([128, 512], f32, tag="hbuf", name="zaf2")
                    nc.vector.tensor_scalar(zaf2, hp, sc5[:, mt:mt + 1],
                                            scalar2=t5p[:, mt:mt + 1], op0=OP.mult, op1=OP.add)
                    nc.vector.scalar_tensor_tensor(feat[:, mt, :], in0=zaf2, scalar=0.2,
                                                   in1=zaf2, op0=OP.mult, op1=OP.max)
                for mt in range(8):
                    hp2 = psum2.tile([128, 512], f32, tag="dps")
                    for ct in range(8):
                        nc.tensor.matmul(hp2, twsb[:, ct, mt * 128:(mt + 1) * 128],
                                         feat[:, ct, :],
                                         start=(ct == 0), stop=(ct == 7))
                    hbt = spool.tile([128, 512], f32, tag="hbuf")
                    nc.vector.tensor_scalar(hbt, hp2, tbc[:, mt:mt + 1], scalar2=None, op0=OP.add)
                    scr1 = spool.tile([128, 512], f32, tag="sqb")
                    nc.scalar.activation(scr1, hbt, ACTF.Copy,
                                         accum_out=acc_ts[:, mt, nk:nk + 1])
                    nc.scalar.activation(scr1, hbt, ACTF.Square,
                                         accum_out=acc_tq[:, mt, nk:nk + 1])
                    nc.vector.tensor_reduce(out=hmax[:, mt, nk:nk + 1], in_=hbt,
                                            axis=AX.X, op=OP.max)
            fin_sb = spool.tile([128, 8, 3], f32, tag="fin_sb")
            nc.vector.tensor_reduce(out=fin_sb[:, :, 0], in_=acc_ts, axis=AX.X, op=OP.add)
            nc.vector.tensor_reduce(out=fin_sb[:, :, 1], in_=acc_tq, axis=AX.X, op=OP.add)
            nc.vector.tensor_reduce(out=fin_sb[:, :, 2], in_=hmax, axis=AX.X, op=OP.max)
            for si in range(3):
                nc.sync.dma_start(out=fin_in[si].rearrange("(t p) -> p t", p=128),
                                  in_=fin_sb[:, :, si])
            nc.gpsimd.collective_compute("AllGather", OP.bypass, replica_groups=ALL8,
                                         ins=[fin_in[:, :].opt()],
                                         outs=[fin_out[:, :, :].opt()])
            allf = spool.tile([128, 8, 8, 3], f32, tag="allf")
            for ri in range(8):
                for si in range(3):
                    nc.sync.dma_start(out=allf[:, ri, :, si],
                                      in_=fin_out[ri, si].rearrange("(t p) -> p t", p=128))
            tws = spool.tile([128, 8, 2], f32, tag="tws")
            for si in range(2):
                nc.vector.tensor_reduce(out=tws[:, :, si],
                                        in_=allf[:, :, :, si].rearrange("p r t -> p t r"),
                                        axis=AX.X, op=OP.add)
            sct, tt = bn_coeffs(tws, float(B * N), tgc, tec, "t")
            P = spool.tile([128, 8, B], f32, tag="P")
            for b in range(B):
                pm = spool.tile([128, 8], f32, tag="pm")
                nc.vector.tensor_max(pm, allf[:, 2 * b, :, 2], allf[:, 2 * b + 1, :, 2])
                for mt in range(8):
                    nc.scalar.activation(P[:, mt, b:b + 1], pm[:, mt:mt + 1], ACTF.Relu,
                                         bias=tt[:, mt:mt + 1], scale=sct[:, mt:mt + 1])

            # ---------------- head ----------------
            def head_layer(Pin, tiles_in, cout, wext, bext, gk, ek, relu=True):
                ot = (cout + 127) // 128
                wsb = spool.tile([128, tiles_in, cout], f32, tag=f"hw{cout}")
                nc.sync.dma_start(out=wsb[:, :, :], in_=wext.rearrange("(t p) o -> p t o", p=128))
                hps = psum.tile([128, max(B * ot, B)], f32, tag="aux")
                for h in range(ot):
                    mh = min(128, cout - h * 128)
                    for ct in range(tiles_in):
                        nc.tensor.matmul(hps[:mh, h * B:(h + 1) * B],
                                         wsb[:, ct, h * 128:h * 128 + mh], Pin[:, ct, :],
                                         start=(ct == 0), stop=(ct == tiles_in - 1))
                bc = spool.tile([128, ot], f32, tag=f"hb{cout}")
                if cout >= 128:
                    nc.sync.dma_start(out=bc[:, :], in_=bext.rearrange("(t p) -> p t", p=128))
                else:
                    nc.sync.dma_start(out=bc[:cout, :], in_=bext[:, None])
                hout = spool.tile([128, ot, B], f32, tag=f"ho{cout}")
                for h in range(ot):
                    mh = min(128, cout - h * 128)
                    nc.vector.tensor_scalar(hout[:mh, h, :], hps[:mh, h * B:(h + 1) * B],
                                            bc[:mh, h:h + 1], scalar2=None, op0=OP.add)
                if gk is None:
                    return hout
                mub = spool.tile([128, ot], f32, tag=f"mb{cout}")
                nc.vector.tensor_reduce(out=mub, in_=hout, axis=AX.X, op=OP.add)
                nc.vector.tensor_scalar(mub, mub, 1.0 / B, scalar2=None, op0=OP.mult)
                hsq = spool.tile([128, ot, B], f32, tag=f"hs{cout}")
                nc.scalar.square(hsq.rearrange("p t b -> p (t b)"),
                                 hout.rearrange("p t b -> p (t b)"))
                vb = spool.tile([128, ot], f32, tag=f"vb{cout}")
                nc.vector.tensor_reduce(out=vb, in_=hsq, axis=AX.X, op=OP.add)
                nc.vector.tensor_scalar(vb, vb, 1.0 / B, scalar2=None, op0=OP.mult)
                mbs = spool.tile([128, ot], f32, tag=f"m2b{cout}")
                nc.vector.tensor_mul(mbs, mub, mub)
                nc.vector.tensor_sub(vb, vb, mbs)
                nc.vector.tensor_scalar_add(vb, vb, EPS)
                nc.scalar.sqrt(vb, vb)
                rsb = spool.tile([128, ot], f32, tag=f"rb{cout}")
                nc.vector.reciprocal(rsb, vb)
                gc = _chan(nc, spool, gk, ot, f"gc{cout}") if cout >= 128 else None
                ec = _chan(nc, spool, ek, ot, f"ec{cout}") if cout >= 128 else None
                scb = spool.tile([128, ot], f32, tag=f"scb{cout}")
                nc.vector.tensor_mul(scb, rsb, gc)
                tb_ = spool.tile([128, ot], f32, tag=f"tbb{cout}")
                nc.vector.tensor_mul(tb_, mub, scb)
                nc.vector.tensor_sub(tb_, ec, tb_)
                ho2 = spool.tile([128, ot, B], f32, tag=f"ho2{cout}")
                for h in range(ot):
                    mh = min(128, cout - h * 128)
                    nc.scalar.activation(ho2[:mh, h, :], hout[:mh, h, :], ACTF.Relu,
                                         bias=tb_[:mh, h:h + 1], scale=scb[:mh, h:h + 1])
                return ho2

            h1 = head_layer(P, 8, 512, f1w_ext, vecs["f1b"], vecs["f1g"], vecs["f1e"])
            h2 = head_layer(h1, 4, 256, f2w_ext, vecs["f2b"], vecs["f2g"], vecs["f2e"])
            h3 = head_layer(h2, 2, 16, f3w_ext, vecs["f3b"], None, None)
            idn = spool.tile([128, 1], f32, tag="idn")
            nc.sync.dma_start(out=idn[:16, :], in_=vecs["idenv"][:, None])
            h3f = spool.tile([128, B], f32, tag="h3f")
            nc.vector.tensor_add(h3f[:12, :], h3[:12, 0, :], idn[:12, :].to_broadcast([12, B]))
            approw = spool.tile([1, 4], f32, tag="approw")
            nc.vector.memset(approw, 0.0)
            nc.vector.memset(approw[:, 3:4], 1.0)
            for b in range(B):
                nc.sync.dma_start(out=out_ext[b, 0:3, :], in_=h3f[:12, b:b + 1])
                nc.sync.dma_start(out=out_ext[b, 3:4, :], in_=approw)
            nc.leave_named_scope("tail", sc_tl[0], False)
    nc.compile()
    return nc


_NC_CACHE = None


def kernel(**inputs):
    global _NC_CACHE
    from concourse.bass_utils import run_bass_kernel_spmd
    x = np.asarray(inputs['x'], np.float32)
    in_maps = []
    for core in range(8):
        b, h = core // 2, core % 2
        xr = np.roll(x[b], -h * NH, axis=1) * SQ2
        m = {"x0": np.ascontiguousarray(xr)}
        for li in range(4):
            W = np.asarray(inputs[f'w{li + 1}'], np.float32)
            C = W.shape[1] // 2
            Wd, Wv = W[:, :C], W[:, C:] - W[:, :C]
            # block inputs are sqrt2-scaled except block 4 output (li==3 input scaled too)
            m[f"wd{li}"] = np.ascontiguousarray(Wd.T / SQ2)
            m[f"wv{li}"] = np.ascontiguousarray(Wv.T / SQ2)
            m[f"bb{li}"] = np.asarray(inputs[f'b{li + 1}'], np.float32)
            m[f"gg{li}"] = np.asarray(inputs[f'g{li + 1}'], np.float32)
            m[f"ee{li}"] = np.asarray(inputs[f'e{li + 1}'], np.float32)
        w5T = np.asarray(inputs['w5'], np.float32).T.copy()
        w5T[:256, :] /= SQ2
        m["w5T"] = np.ascontiguousarray(w5T)
        m["twT"] = np.ascontiguousarray(np.asarray(inputs['tw'], np.float32).T)
        for nm in ("b5", "g5", "e5", "tb", "tg", "te", "f1b", "f1g", "f1e",
                   "f2b", "f2g", "f2e"):
            m[nm] = np.asarray(inputs[nm], np.float32)
        f3bp = np.zeros(16, np.float32); f3bp[:12] = np.asarray(inputs['f3b'], np.float32)
        m["f3b"] = f3bp
        idv = np.zeros(16, np.float32); idv[[0, 5, 10]] = 1.0
        m["idenv"] = idv
        m["f1wT"] = np.ascontiguousarray(np.asarray(inputs['f1w'], np.float32).T)
        m["f2wT"] = np.ascontiguousarray(np.asarray(inputs['f2w'], np.float32).T)
        f3T = np.asarray(inputs['f3w'], np.float32).T  # [256, 12]
        f3Tp = np.zeros((256, 16), np.float32)
        f3Tp[:, :12] = f3T
        m["f3wT"] = f3Tp
        in_maps.append(m)
    if _NC_CACHE is None:
        _NC_CACHE = build()
    kernel.last_in_maps = in_maps
    import os
    trace = bool(os.environ.get("KERNEL_TRACE"))
    res = run_bass_kernel_spmd(_NC_CACHE, in_maps, core_ids=list(range(8)), trace=trace)
    kernel.last_result = res
    return res.results[0]["out"].reshape(B, 4, 4)
